# revision 1
# baseline (speedup 1.0000x reference)
"""Causal self-attention Trainium2 Bass kernel (v3).

Problem: B=2, N=2048, D=1024, H=16 heads, DH=64 (fp32).
  kqv = einsum('bnd,hed->bhne', x, Wqkv) + bqkv   (chunk order k, q, v)
  scores = q @ k^T / 8, causal mask, softmax
  sa = attn @ v, concat heads, out = sa @ Wproj.T + bproj

Sharding (8 cores): data-parallel over B (2) x tensor-parallel over heads
(4 heads/core).  Each core computes its 4 heads' contribution to the proj
output for its batch; the host sums the 4 partials per batch and adds
bproj (the "all-reduce after proj" done host-side during unsharding).

v3 changes over v2 (181.3us):
  - QKV merged into the attention pipeline: only v01/k01/q01 run as a
    serial prefix; the k23/q23/v23 projection matmuls are paced into
    the heads-0/1 attention pair pipeline as auxiliary PE work, so
    there is no QKV->attention transition bubble and the PE never
    idles long enough for the HAM clock gate to re-throttle.
  - Unit order is two head-passes (h01 then h23), qb DESCENDING inside
    each pass: the all-diagonal qb=0 units (lowest PE duty, v2's cold-
    matmul hotspot) now run with proj/QKV aux work interleaved.
  - V transpose moved from PE+ScalarE to the DMA xbar
    (dma_start_transpose, 32x [128,128] SBUF->SBUF): frees ~9us of PE
    and ~14us of ScalarE; the warmup transpose spin is dropped (HW
    evidence: transpose-mode does not engage the HAM fast clock).
  - exp table preloaded by a dummy ACTIVATE at kernel start (~2.7us
    off the first real exp).
  - PSUM: qkv-accum 2 + score-pairs 4 + PV-accum 2 = 8 banks; proj
    reuses the qkv-accum slots (QKV is finished by proj time), and the
    final 4 proj tiles also borrow the drained score-pair slots so the
    tail has 4 buffers and no copy-wait stalls.
"""

import numpy as np
from contextlib import ExitStack

B, N, D, H = 2, 2048, 1024, 16
DH = 64
NH = 4                    # heads per core
E = NH * 3 * DH           # 768 local qkv output dim
ET = E // 128             # 6 e-tiles: [k01 k23 | q01 q23 | v01 v23]
DT = D // 128             # 8 d-tiles (contraction)
NBS = 512                 # n block size (moving operand width)
NB = N // NBS             # 4 n blocks
MTS = 128                 # m tile size (key-axis tile)
MT = N // MTS             # 16 m tiles
KT = NH * DH // 128       # 2 proj contraction tiles (256 local d_in)

USE_XBAR = False          # V transpose via DMA xbar (False: PE transpose)

_CACHE = {}


def _build_nc(use_xbar=USE_XBAR):
    import concourse.mybir as mybir
    import concourse.tile as tile
    from concourse import bacc

    f32 = mybir.dt.float32
    bf16 = mybir.dt.bfloat16
    EXP = mybir.ActivationFunctionType.Exp

    nc = bacc.Bacc("TRN2")
    xT_d = nc.dram_tensor("xT", [128, DT * N], bf16, kind="ExternalInput")
    wT_d = nc.dram_tensor("wT", [128, ET * DT * 128], bf16,
                          kind="ExternalInput")
    bq_d = nc.dram_tensor("bq", [128, ET], f32, kind="ExternalInput")
    wpT_d = nc.dram_tensor("wpT", [128, KT * D], bf16, kind="ExternalInput")
    out_d = nc.dram_tensor("outp", [N, D], f32, kind="ExternalOutput")

    xTr = xT_d.rearrange("p (t n) -> p t n", t=DT)
    wTr = wT_d.rearrange("p (e t j) -> p e t j", e=ET, t=DT)
    wpTr = wpT_d.rearrange("p (k f) -> p k f", k=KT)

    with tile.TileContext(nc) as tc, ExitStack() as ctx:
        const = ctx.enter_context(tc.tile_pool(name="const", bufs=1))
        xp = ctx.enter_context(tc.tile_pool(name="xw", bufs=1))
        wsp = ctx.enter_context(tc.tile_pool(name="wst", bufs=3))
        qps = ctx.enter_context(tc.tile_pool(name="qps", bufs=2,
                                             space="PSUM"))
        sps = ctx.enter_context(tc.tile_pool(name="sps", bufs=2,
                                             space="PSUM"))
        pts = ctx.enter_context(tc.tile_pool(name="pts", bufs=8))
        sapp = ctx.enter_context(tc.tile_pool(name="sap", bufs=2,
                                              space="PSUM"))
        rrp = ctx.enter_context(tc.tile_pool(name="rrp", bufs=3))
        ost = ctx.enter_context(tc.tile_pool(name="ost", bufs=4))

        # preload the exp table set with a dummy ACTIVATE (~2.7us once);
        # the result lands in saT which is fully overwritten by the
        # normalization muls before any proj matmul reads it
        dum0 = const.tile([128, 1], bf16)
        nc.vector.memset(dum0, 0.0)

        bq = const.tile([128, ET], f32)
        wpT = const.tile([128, KT, D], bf16)
        kqv = const.tile([128, 4, N], bf16)   # [k01 k23 v01 v23] e-tiles
        qpad = []
        for h in range(NH):
            t = const.tile([128, N], bf16, name=f"qpad{h}")
            qpad.append(t)
            po = (h % 2) * 64
            nc.vector.memset(t[64 - po:128 - po, :], 0.0)
        # V operand per head: cols 0:64 = V, cols 64:128 = ones (the
        # ones make the PV matmul emit the softmax denominator free)
        vaug = const.tile([128, NH, MT, 128], bf16)
        nc.vector.memset(vaug, 1.0)
        saT = const.tile([128, KT, N], bf16)  # sa^T, local d_in partitions
        nc.scalar.activation(saT[:, 0, 0:1], dum0, EXP)
        # bacc pre-registers const APs; the BIR verifier rejects unread
        # SBUF - give the unused ones readers (slots overwritten later)
        for i, key in enumerate([(f32, 1.0), (bf16, 1.0),
                                 (mybir.dt.uint8, 127)]):
            nc.vector.tensor_copy(saT[:, 0, i + 1:i + 2],
                                  nc.const_aps.aps[key])

        if not use_xbar:
            ones = const.tile([128, 128], bf16)
            nc.gpsimd.memset(ones, 1.0)
            ident = const.tile([128, 128], bf16)
            nc.gpsimd.affine_select(
                ident, ones, pattern=[[-1, 128]], base=0,
                channel_multiplier=1,
                compare_op=mybir.AluOpType.is_equal, fill=0.0)

        # ---------------- input DMA schedule ----------------
        # sync ring: first QKV weight tile, then even xT chunks (+ out
        # DMAs later); scalar ring: odd xT chunks (+ xbar transposes);
        # gpsimd ring: bias, remaining QKV weights, proj weights.
        wst_tiles = {}
        wst_tiles[4] = wsp.tile([128, DT, 128], bf16, name="wst4")
        nc.sync.dma_start(out=wst_tiles[4], in_=wTr[:, 4, :, :])
        xT = xp.tile([128, DT, N], bf16)
        for dt in range(DT):
            eng = nc.sync if dt % 2 == 0 else nc.scalar
            eng.dma_start(out=xT[:, dt, :], in_=xTr[:, dt, :])
        nc.gpsimd.dma_start(out=bq, in_=bq_d[:, :])

        def fetch_wst(et):
            wst_tiles[et] = wsp.tile([128, DT, 128], bf16, name=f"wst{et}")
            nc.gpsimd.dma_start(out=wst_tiles[et], in_=wTr[:, et, :, :])

        # ---------------- QKV building blocks ----------------
        def qkv_mm_chunk(et, nbh, dt, pss):
            for nb in (2 * nbh, 2 * nbh + 1):
                nc.tensor.matmul(
                    pss[nb % 2],
                    lhsT=wst_tiles[et][:, dt, :],
                    rhs=xT[:, dt, nb * NBS:(nb + 1) * NBS],
                    start=(dt == 0),
                    stop=(dt == DT - 1),
                )

        def qkv_bias(et, nb, ps):
            nbs = slice(nb * NBS, (nb + 1) * NBS)
            if et in (2, 3):          # q: split per head into qpad
                for j in range(2):
                    hh = 2 * (et - 2) + j
                    nc.vector.tensor_scalar_add(
                        out=qpad[hh][64 * j:64 * j + 64, nbs],
                        in0=ps[64 * j:64 * j + 64, :],
                        scalar1=bq[64 * j:64 * j + 64, et:et + 1],
                    )
            else:                     # k and v: packed 2-head tiles
                dst = et if et < 2 else et - 2
                nc.vector.tensor_scalar_add(
                    out=kqv[:, dst, nbs],
                    in0=ps,
                    scalar1=bq[:, et:et + 1],
                )

        def qkv_group_items(et, fetch=True):
            """Closures for one e-tile's projection (trigger, MMs, bias)."""
            items = [lambda et=et: fetch_wst(et)] if fetch else []
            for nbh in range(2):
                holder = {}

                def alloc(et=et, nbh=nbh, holder=holder):
                    holder["pss"] = [
                        qps.tile([128, NBS], f32, tag="qkvps",
                                 name=f"qkvps{et}_{nbh}_{i}")
                        for i in range(2)]
                items.append(alloc)
                for dt in range(DT):
                    items.append(lambda et=et, nbh=nbh, dt=dt, holder=holder:
                                 qkv_mm_chunk(et, nbh, dt, holder["pss"]))
                for i in range(2):
                    items.append(lambda et=et, nbh=nbh, i=i, holder=holder:
                                 qkv_bias(et, 2 * nbh + i, holder["pss"][i]))
            return items

        def vtrans_items(vt):
            """V transpose: kqv v-tile -> vaug [m, dh] layout."""
            items = []
            if use_xbar:
                def xb(mt, vt=vt):
                    nc.scalar.dma_start(
                        out=vaug[:, 2 * vt, mt, 0:DH],
                        in_=kqv[:, 2 + vt, mt * MTS:(mt + 1) * MTS],
                        transpose=True)
                for mt0 in range(0, MT, 4):
                    def grp(mt0=mt0, vt=vt):
                        for mt in range(mt0, mt0 + 4):
                            xb(mt, vt)
                    items.append(grp)
            else:
                def pe_tr(mt, vt=vt):
                    pv = qps.tile([128, 128], bf16, tag="qkvps", name="pv")
                    with nc.allow_low_precision(reason="transpose pass"):
                        nc.tensor.transpose(
                            pv, kqv[:, 2 + vt, mt * MTS:(mt + 1) * MTS],
                            ident)
                    for j in range(2):
                        nc.scalar.copy(vaug[:, 2 * vt + j, mt, 0:DH],
                                       pv[:, 64 * j:64 * j + 64])
                for mt in range(MT):
                    items.append(lambda mt=mt: pe_tr(mt))
            return items

        # ---------------- serial prefix: v01, k01, q01 ----------------
        for it in qkv_group_items(4, fetch=False):  # wst4 pre-fetched
            it()
        for it in vtrans_items(0):
            it()
        for et in (0, 2):
            for it in qkv_group_items(et):
                it()

        # aux PE/DVE work paced into the heads-0/1 attention pipeline
        aux = []
        for et in (5, 1, 3):
            aux.extend(qkv_group_items(et))
            if et == 5:
                aux.extend(vtrans_items(1))
        aux.append(lambda: nc.gpsimd.dma_start(out=wpT, in_=wpTr))

        # ---------------- attention + projection pipeline ----------------
        units = []
        for hpass in ((0, 1), (2, 3)):
            for qb in (3, 2, 1, 0):
                for h in hpass:
                    units.append((qb, h))
        gp = []
        for ui, (qb, h) in enumerate(units):
            for mp in range(2 * qb + 2):
                gp.append((ui, mp))
        TOT = len(gp)                 # 80

        sap_tiles = {}
        pt_tiles = {}
        state = {"s": 0}

        def emit_S(g):
            ui, mp = gp[g]
            qb, h = units[ui]
            qmv = qpad[h][:, qb * NBS:(qb + 1) * NBS]
            kt_tile = kqv[:, h // 2, :]
            sp = sps.tile([128, 2, NBS], f32, tag="sp", name="sp")
            for j in range(2):
                mt = 2 * mp + j
                nc.tensor.matmul(
                    sp[:, j, :],
                    lhsT=kt_tile[:, mt * MTS:(mt + 1) * MTS],
                    rhs=qmv,
                    start=True, stop=True,
                )
            diag = mp >= 2 * qb
            pt = pts.tile([128, 2, NBS], bf16,
                          tag="ptd" if diag else "pt", name="pt")
            nc.scalar.activation(pt, sp, EXP, scale=0.125)
            if diag:
                # causal mask on GpSimd: keep where j >= i + 128*(rel+a)
                rel = 2 * mp - 4 * qb
                ptm = pts.tile([128, 2, NBS], bf16, tag="ptm", name="ptm")
                nc.gpsimd.affine_select(
                    ptm, pt, pattern=[[-MTS, 2], [1, NBS]],
                    base=-MTS * rel, channel_multiplier=-1,
                    compare_op=mybir.AluOpType.is_ge, fill=0.0)
                pt = ptm
            pt_tiles[g] = pt

        def pump_S(upto):
            while state["s"] <= min(upto, TOT - 1):
                emit_S(state["s"])
                state["s"] += 1

        def emit_PV(g):
            ui, mp = gp[g]
            qb, h = units[ui]
            nmt = 4 * qb + 4
            if ui not in sap_tiles:
                sap_tiles[ui] = sapp.tile([128, NBS], f32, name="sap")
            sap = sap_tiles[ui]
            for j in range(2):
                mt = 2 * mp + j
                nc.tensor.matmul(
                    sap,
                    lhsT=vaug[:, h, mt, :],
                    rhs=pt_tiles[g][:, j, :],
                    start=(mt == 0), stop=(mt == nmt - 1),
                )
            del pt_tiles[g]

        def emit_norm(ui):
            qb, h = units[ui]
            sap = sap_tiles[ui]
            # HW constraints (micro-tested): reciprocal_approx_fast only
            # works at base partition 0, and 2-input DVE ops need equal
            # input base partitions - shift the denom rows down first.
            den = rrp.tile([128, NBS], f32, tag="den", name="den")
            nc.vector.tensor_copy(den[0:DH, :], sap[DH:128, :])
            rr = rrp.tile([128, NBS], f32, tag="rr", name="rr")
            nc.vector.reciprocal_approx_fast(
                out=rr[0:DH, :], in_=den[0:DH, :])
            nc.vector.tensor_mul(
                saT[(h % 2) * DH:(h % 2) * DH + DH, h // 2,
                    qb * NBS:(qb + 1) * NBS],
                sap[0:DH, :], rr[0:DH, :])

        def emit_proj(nt, borrow_sp=False):
            if borrow_sp:
                po = sps.tile([128, 2, NBS], f32, tag="sp", name="sp")
                po0, po1 = po[:, 0, :], po[:, 1, :]
            else:
                po0 = qps.tile([128, NBS], f32, tag="qkvps", name="po0")
                po1 = qps.tile([128, NBS], f32, tag="qkvps", name="po1")
            for kt in range(KT):
                lt = saT[:, kt, nt * 128:(nt + 1) * 128]
                nc.tensor.matmul(po0, lhsT=lt, rhs=wpT[:, kt, 0:NBS],
                                 start=(kt == 0), stop=(kt == KT - 1))
                nc.tensor.matmul(po1, lhsT=lt, rhs=wpT[:, kt, NBS:D],
                                 start=(kt == 0), stop=(kt == KT - 1))
            ot = ost.tile([128, D], f32, name="ot")
            if borrow_sp:
                nc.vector.tensor_copy(
                    ot.rearrange("p (a b) -> p a b", a=2), po)
            else:
                nc.vector.tensor_copy(ot[:, 0:NBS], po0)
                nc.vector.tensor_copy(ot[:, NBS:], po1)
            nc.sync.dma_start(out=out_d[nt * 128:(nt + 1) * 128, :], in_=ot)

        pump_S(1)
        for g in range(TOT):
            ui, mp = gp[g]
            qb, h = units[ui]
            diag = mp >= 2 * qb
            pump_S(g + 3 if diag else g + 2)
            emit_PV(g)
            budget = 3 if diag else 2
            while aux and budget > 0:
                aux.pop(0)()
                budget -= 1
            if mp == 2 * qb + 1:      # last pair of this unit
                emit_norm(ui)
                del sap_tiles[ui]
                if h == 3:            # this qb's saT slab is complete
                    aux.extend(
                        lambda nt=4 * qb + k: emit_proj(nt)
                        for k in range(4))
        while aux:
            aux.pop(0)()
        # tail: proj for the last finished qb (qb=0) with 4 buffers
        for k in range(4):
            emit_proj(k, borrow_sp=(k >= 2))

    nc.compile()
    return nc


def _host_inputs(x, Wqkv, bqkv, Wproj):
    """Per-core input maps (host-side sharding + relayout, bf16 cast).

    All tensors are packed partition-major so every DMA descriptor is a
    contiguous >=2KB row chunk.
    """
    import ml_dtypes
    bf16 = ml_dtypes.bfloat16

    in_maps = []
    for c in range(8):
        b, hg = c // NH, c % NH
        h0 = hg * NH
        # xT[p, dt, n] = x[b][n, dt*128+p]
        xT = np.ascontiguousarray(
            x[b].T.reshape(DT, 128, N).transpose(1, 0, 2)
            .reshape(128, DT * N)).astype(bf16)
        # e-axis order: [all-k (NH*DH), all-q, all-v] so each head's k/q/v
        # slices share a base partition (matmul operand constraint).
        wq = Wqkv[h0:h0 + NH].reshape(NH, 3, DH, D)
        wT = wq.transpose(1, 0, 2, 3).reshape(E, D).T          # [D, E]
        # wT2[p, et, dt, j] = wT[dt*128+p, et*128+j]
        wT2 = np.ascontiguousarray(
            wT.reshape(DT, 128, ET, 128).transpose(1, 2, 0, 3)
            .reshape(128, ET * DT * 128)).astype(bf16)
        bqc = bqkv[h0:h0 + NH].reshape(NH, 3, DH).transpose(1, 0, 2) \
            .reshape(E)
        bq2 = np.ascontiguousarray(
            bqc.reshape(ET, 128).T).astype(np.float32)         # [128, ET]
        wpT = Wproj[:, h0 * DH:(h0 + NH) * DH].T               # [256, D]
        wpT2 = np.ascontiguousarray(
            wpT.reshape(KT, 128, D).transpose(1, 0, 2)
            .reshape(128, KT * D)).astype(bf16)
        in_maps.append({"xT": xT, "wT": wT2, "bq": bq2, "wpT": wpT2})
    return in_maps


def _get_nc():
    if "nc" not in _CACHE:
        _CACHE["nc"] = _build_nc()
    return _CACHE["nc"]


def run_on_hw(in_maps, trace=False, **kw):
    from concourse.bass_utils import run_bass_kernel_spmd
    nc = _get_nc()
    return run_bass_kernel_spmd(
        nc, in_maps, core_ids=list(range(8)), trace=trace, **kw)


def kernel(**inputs):
    x = np.asarray(inputs["x"], dtype=np.float32)
    Wqkv = np.asarray(inputs["Wqkv"], dtype=np.float32)
    bqkv = np.asarray(inputs["bqkv"], dtype=np.float32)
    Wproj = np.asarray(inputs["Wproj"], dtype=np.float32)
    bproj = np.asarray(inputs["bproj"], dtype=np.float32)

    in_maps = _host_inputs(x, Wqkv, bqkv, Wproj)
    res = run_on_hw(in_maps).results

    out = np.zeros((B, N, D), dtype=np.float32)
    for b in range(B):
        acc = res[b * NH + 0]["outp"].astype(np.float32)
        for g in range(1, NH):
            acc = acc + res[b * NH + g]["outp"]
        out[b] = acc + bproj[None, :]
    return out

